# revision 36
# baseline (speedup 1.0000x reference)
"""Trainium2 Bass kernel for nn_Encoder_conv_mlp (GNN message passing encoder).

Reference computation (per graph batch):
    h1 = relu(segsum(x[src]->dst) @ W1_rel.T + x @ W1_root.T + b1)
    h2 = relu(segsum(h1[src]->dst) @ W2_rel.T + h1 @ W2_root.T + b2)
    hb = h2.reshape(bs, 64*256)
    mu = hb @ Wmu.T + bmu ; logvar = hb @ Wlv.T + blv

Sharding: data-parallel over graphs. 512 graphs / 8 cores = 64 graphs
(4096 nodes, 65536 edges) per core. Edges never cross graphs, so each
core is fully independent; weights are replicated and the host simply
concatenates the per-core [64, 256] outputs.

Message passing is done as dense matmuls: the host builds, for every
2-graph block (128 nodes), an adjacency count matrix A2T[s, d] =
#edges(src=s -> dst=d). On device, aggregation is A2T contracted over
the src-node partition dim. Two matmul "families" avoid all transposes:
  - activations stationary (lhsT) + weights moving  -> node-major out
  - weights stationary (lhsT) + activations moving  -> feature-major out
Layer outputs are kept feature-major; the rel-projection (node-major) is
an intermediate only. All matmul operands are bf16 (fp32 PSUM accum).

The [16384, 256] readout weight [Wmu.T | Wlv.T] (8.4 MB bf16 per core)
is prefetched into SBUF while the conv layers run. Inputs are loaded
into per-chunk SBUF tiles so compute starts as soon as its chunk lands
(whole-tile deps would stall the PE on the full transfer), and h1 is
split per (ko, group) so layer 2 pipelines behind layer 1.

Further scheduling details: the f32 biases and the w1 weights ride
packed inside the xw/w2 bf16 input tensors (f32 values via bitcast
views on device) so the serial per-DMA launch overhead is paid fewer
times and the first matmul's dependencies arrive in a single transfer;
a short stream of discarded warm-up matmuls keeps the PE clock ramp
(HAM) busy while the first input DMAs land; and layer 2 runs all
rel-projections first, then the whole mo=0 output pass before mo=1, so
h2's first feature half (which gates the readout) completes while the
PE still has a full pass of work queued.
"""
import sys

if "/opt/trn_rl_repo" not in sys.path:
    sys.path.insert(0, "/opt/trn_rl_repo")

import numpy as np
import ml_dtypes

N_NODES = 64
BS = 512
IN_F = 128
HID = 256
LAT = 128
N_CORES = 8
G_PER = BS // N_CORES          # 64 graphs per core
NODES_PER = G_PER * N_NODES    # 4096 nodes per core
BLOCKS = NODES_PER // 128      # 32 two-graph blocks per core
GROUPS = NODES_PER // 512      # 8 512-node groups per core
KT = (N_NODES * HID) // 128    # 128 readout contraction tiles

BF16 = ml_dtypes.bfloat16
F8E3 = ml_dtypes.float8_e3m4
S3 = 512.0          # wro is stored as fp8-e3m4 * S3; readout evicts with 1/S3

_PROGRAM = None


def _build_program():
    import concourse.bacc as bacc
    import concourse.mybir as mybir
    import concourse.tile as tile

    nc = bacc.Bacc("TRN2", target_bir_lowering=False, debug=False,
                   num_devices=N_CORES)
    BF = mybir.dt.bfloat16
    F32 = mybir.dt.float32
    E3 = mybir.dt.float8e3

    xw = nc.dram_tensor("xw", [128, 1032], BF, kind="ExternalInput").ap()
    # nma: per 2-graph block, [x node-major (128) | a2t counts (128)] pairs —
    # L1 aggregates x directly (agg-first), so each block's pair arrives in
    # one contiguous chunk.
    nma = nc.dram_tensor("nma", [128, BLOCKS * 256], E3, kind="ExternalInput").ap()
    # feature-major x for groups 1-7, fp8-e3m4 (x scaled by 2; group 0 rides
    # in the bf16 lead, also pre-scaled by 2)
    xf8 = nc.dram_tensor("xf8", [128, 3584], E3, kind="ExternalInput").ap()
    # w2 carries [W2 packs | 128x128 bf16 identity | bmu/blv per-latent f32]
    w2 = nc.dram_tensor("w2", [128, 1536], BF, kind="ExternalInput").ap()
    # readout weights in fp8-e3m4 (scaled by S3): halves the dominant DMA
    wro = nc.dram_tensor("wro", [128, KT * 256], E3, kind="ExternalInput").ap()
    out = nc.dram_tensor("out", [G_PER, 256], F32, kind="ExternalOutput").ap()

    Relu = mybir.ActivationFunctionType.Relu

    with tile.TileContext(nc) as tc:
        with (
            tc.tile_pool(name="const", bufs=1) as const,
            tc.tile_pool(name="hr", bufs=20) as hr_pool,
            tc.tile_pool(name="psum_hr", bufs=3, space="PSUM") as psum_hr,
            tc.tile_pool(name="psum_fm", bufs=3, space="PSUM") as psum_fm,
            tc.tile_pool(name="psum_ro", bufs=1, space="PSUM") as psum_ro,
            tc.tile_pool(name="psum_t", bufs=1, space="PSUM") as psum_t,
        ):
            # Per-chunk tiles so each consumer depends only on its chunk's DMA.
            lead_sb = const.tile([128, 1032], BF, tag="lead_sb")
            xT0b_sb = const.tile([128, 512], E3, tag="xT0b_sb")
            xT_sb = [const.tile([128, 1024], E3, name=f"xT{i}", tag=f"xT{i}")
                     for i in range(1, 4)]
            # nma per-group tiles; group 0 is split so block 0's (x_nm|a2t)
            # pair lands in the smallest possible first transfer.
            nm0a_sb = const.tile([128, 256], E3, tag="nm0a_sb")
            nm0b_sb = const.tile([128, 768], E3, tag="nm0b_sb")
            nm_sb = [const.tile([128, 1024], E3, name=f"nm{g}", tag=f"nm{g}")
                     for g in range(1, GROUPS)]
            w2_sb = const.tile([128, 1536], BF, tag="w2_sb")
            wro_sb = [const.tile([128, 4096], E3, name=f"wro{i}", tag=f"wro{i}") for i in range(8)]
            # h1 split per (ko, group) for L1->L2 pipelining; h2 per ko chunk.
            h1_sb = [[const.tile([128, 512], BF, name=f"h1_{ko}_{g}", tag=f"h1_{ko}_{g}")
                      for g in range(GROUPS)] for ko in range(2)]
            h2_sb = [const.tile([128, NODES_PER], BF, name=f"h2_{fo}", tag=f"h2_{fo}")
                     for fo in range(2)]

            def nm_chunk(b):           # (x_nm | a2t) [128, 256] pair, block b
                if b == 0:
                    return nm0a_sb[:, 0:256]
                if b < 4:
                    return nm0b_sb[:, (b - 1) * 256:b * 256]
                return nm_sb[b // 4 - 1][:, (b % 4) * 256:(b % 4 + 1) * 256]

            def x_nm_blk(b):           # node-major x block [128 node, 128 f]
                return nm_chunk(b)[:, 0:128]

            def a2t_blk(b):            # [128, 128] adjacency for block b
                return nm_chunk(b)[:, 128:256]

            # DMA issue order = priority order for the head of the kernel.
            # Block 0's aggregation pair goes first (it gates the very first
            # real matmul), then the lead transfer (w1 + biases + group 0's
            # feature-major x), then x/nma chunks in consumption order ahead
            # of w2 and the big readout-weight stream.
            nc.sync.dma_start(nm0a_sb[:], nma[:, 0:256])
            nc.sync.dma_start(nm0b_sb[:], nma[:, 256:1024])
            nc.sync.dma_start(lead_sb[:], xw[:, 0:1032])
            nc.sync.dma_start(nm_sb[0][:], nma[:, 1024:2048])
            nc.sync.dma_start(xT0b_sb[:], xf8[:, 0:512])
            for i in range(1, 4):
                nc.sync.dma_start(nm_sb[2 * i - 1][:],
                                  nma[:, 2 * i * 1024:(2 * i + 1) * 1024])
                nc.sync.dma_start(nm_sb[2 * i][:],
                                  nma[:, (2 * i + 1) * 1024:(2 * i + 2) * 1024])
                nc.sync.dma_start(xT_sb[i - 1][:],
                                  xf8[:, i * 1024 - 512:(i + 1) * 1024 - 512])
            nc.sync.dma_start(w2_sb[:], w2[:])
            # w1 + biases ride packed inside lead/w2 (bitcast views for f32)
            w1_sb = lead_sb[:, 0:520]
            b12_sb = lead_sb[:, 512:520].bitcast(F32)
            # [128, 128] f32 identity for the f32 PE-transposes
            ident_sb = w2_sb[:, 1024:1280].bitcast(F32)
            brow_sb = w2_sb[0:1, 1280:1536]             # [1, 256] bf16 bmu|blv
            for i in range(8):
                nc.sync.dma_start(wro_sb[i][:], wro[:, i * 4096:(i + 1) * 4096])

            # PE pre-warm: dummy matmuls on memset data keep the PE busy from
            # ~1.1us so the clock ramp (HAM) completes before the first real
            # matmul arrives behind the input DMAs (~3.3us); the count is
            # tuned so the warm stream ends just as the real one begins.
            # Results are discarded; the psum slot is reused by the readout.
            N_WARM = 16
            ones_sb = const.tile([1, 256], BF, tag="ones_sb")
            nc.vector.memset(ones_sb[:], 1.0)
            # ro_big hosts the warmup target, then the readout accumulator —
            # one psum bank serves both phases.
            ro_big = psum_ro.tile([128, 128], F32, tag="pro")
            warm = ro_big[:, 0:128]
            # pt gets its own bank: the per-latent biases are pre-loaded into
            # it by a rank-1 matmul mid-kernel, and the final transposes
            # accumulate on top — so it must not share a psum zero-region
            # with the readout accumulator.
            pt = psum_t.tile([G_PER, 256], F32, tag="pt")
            for i in range(N_WARM):
                nc.tensor.matmul(warm[:], lhsT=ones_sb[:, 128:256],
                                 rhs=ones_sb[:, 0:128],
                                 start=(i == 0), stop=(i == N_WARM - 1))

            def x_cols(c0, c1):        # feature-major x slice [128, c1-c0]
                g = c0 // 512
                if g == 0:
                    assert c1 <= 512
                    return lead_sb[:, 520 + c0:520 + c1]
                if g == 1:
                    assert c1 <= 1024
                    return xT0b_sb[:, c0 - 512:c1 - 512]
                i = (c0 - 1024) // 1024
                assert c1 - 1024 <= (i + 1) * 1024
                return xT_sb[i][:, c0 - 1024 - i * 1024:c1 - 1024 - i * 1024]

            # ---- Layer 1: aggregate-first ----
            # agg_x = A @ x per block (x node-major as stationary, a2t
            # moving), evicted to SBUF; then h1 = relu(W1relT-proj(agg_x) +
            # W1rootT-proj(x_fm) + b1) lands feature-major in one psum pass.
            # Aggregating the 128-wide x (instead of the 256-wide x@W1rel)
            # halves L1's aggregation matmul rows vs project-then-aggregate.
            # Software-pipelined: the aggregation for group g+1 is emitted
            # before group g's projection pass, so the PE chews on agg(g+1)
            # while g's psum->SBUF eviction (DVE) is still in flight.
            def emit_agg(grp):
                pag = psum_hr.tile([128, 512], F32, name="ph", tag="ph")
                for blk in range(4):
                    b = grp * 4 + blk
                    nc.tensor.matmul(
                        pag[:, blk * 128:(blk + 1) * 128],
                        lhsT=x_nm_blk(b), rhs=a2t_blk(b),
                        start=True, stop=True, skip_group_check=True,
                    )
                aggx = hr_pool.tile([128, 512], BF, name="aggx", tag="aggx")
                nc.vector.tensor_copy(aggx[:], pag[:])
                return aggx

            def emit_l1fm(grp, aggx):
                xg = x_cols(grp * 512, (grp + 1) * 512)
                for mo in range(2):
                    pf = psum_fm.tile([128, 512], F32, name="pf", tag="pf")
                    nc.tensor.matmul(
                        pf[:], lhsT=w1_sb[:, mo * 128:(mo + 1) * 128],
                        rhs=aggx[:], start=True, stop=False,
                        skip_group_check=True,
                    )
                    nc.tensor.matmul(
                        pf[:], lhsT=w1_sb[:, 256 + mo * 128:256 + (mo + 1) * 128],
                        rhs=xg, start=False, stop=True,
                        skip_group_check=True,
                    )
                    nc.scalar.activation(
                        h1_sb[mo][grp][:], pf[:], Relu,
                        bias=b12_sb[:, mo:mo + 1], scale=0.5,
                    )

            aggxs = [emit_agg(0), emit_agg(1)]
            for grp in range(GROUPS):
                if grp + 2 < GROUPS:
                    aggxs.append(emit_agg(grp + 2))
                emit_l1fm(grp, aggxs[grp])

            # ---- Layer 2 ----
            for layer in [1]:
                n_ko = 2
                act_cols = lambda ko, c0, c1: (
                    h1_sb[ko][c0 // 512][:, c0 % 512:c0 % 512 + (c1 - c0)])
                w_rel = lambda ko: w2_sb[:, ko * 512:ko * 512 + 256]
                w_root = lambda ko, mo: w2_sb[:, ko * 512 + 256 + mo * 128:
                                              ko * 512 + 256 + (mo + 1) * 128]
                bias_col = 2

                def emit_hr(grp):
                    # two blocks share one [128,512] psum tile (same bank
                    # footprint as a padded [128,256]) so one DVE copy evicts
                    # both -> half the copy count, ~4us less DVE busy
                    hrs = []
                    for pair in range(2):
                        ph = psum_hr.tile([128, 512], F32, name="ph", tag="ph")
                        for sub in range(2):
                            b = grp * 4 + pair * 2 + sub
                            for ko in range(n_ko):
                                nc.tensor.matmul(
                                    ph[:, sub * 256:(sub + 1) * 256],
                                    lhsT=act_cols(ko, b * 128, (b + 1) * 128),
                                    rhs=w_rel(ko),
                                    start=(ko == 0), stop=(ko == n_ko - 1),
                                    skip_group_check=True,
                                )
                        hr = hr_pool.tile([128, 512], BF)
                        nc.vector.tensor_copy(hr[:], ph[:])
                        hrs.append(hr)
                    return hrs

                def emit_fm(grp, mo, hrs):
                    pf = psum_fm.tile([128, 512], F32, name="pf", tag="pf")
                    for ko in range(n_ko):
                        nc.tensor.matmul(
                            pf[:],
                            lhsT=w_root(ko, mo),
                            rhs=act_cols(ko, grp * 512, (grp + 1) * 512),
                            start=(ko == 0), stop=False,
                            skip_group_check=True,
                        )
                    for blk in range(4):
                        b = grp * 4 + blk
                        nc.tensor.matmul(
                            pf[:, blk * 128:(blk + 1) * 128],
                            lhsT=hrs[blk // 2][:, (blk % 2) * 256 + mo * 128:
                                               (blk % 2) * 256 + (mo + 1) * 128],
                            rhs=a2t_blk(b),
                            start=False, stop=(blk == 3),
                            skip_group_check=True,
                        )
                    dst = h2_sb[mo][:, grp * 512:(grp + 1) * 512]
                    nc.scalar.activation(
                        dst, pf[:], Relu,
                        bias=b12_sb[:, bias_col + mo:bias_col + mo + 1],
                    )

                # ---- Readout accumulation, interleaved with L2's tail ----
                # Orientation: latent on the PSUM partition dim (128, full)
                # and graphs on the free dim (64) — half the matmul rows of
                # the graphs-on-partition orientation. wro is the fp8-e3m4
                # stationary operand; h2 (bf16) moves. pro[:, 0:64]
                # accumulates mu, pro[:, 64:128] logvar; one group spans both.
                pro = ro_big[:, 0:128]
                ro_emitted = 0

                def emit_ro(n_kts):
                    # fo=0 k-tiles first (they only need the mo=0 pass of
                    # h2), fo=1 after; interleaving fo=0 emission into the
                    # mo=1 projection pass keeps the PE fed while the last
                    # h2 evictions drain.
                    nonlocal ro_emitted
                    kts = [kt for kt in range(KT) if kt % 2 == 0] + \
                          [kt for kt in range(KT) if kt % 2 == 1]
                    for i in range(ro_emitted, min(ro_emitted + n_kts, KT)):
                        kt = kts[i]
                        n, fo = kt // 2, kt % 2
                        rhs = h2_sb[fo][:, n:n + (G_PER - 1) * N_NODES + 1:
                                        N_NODES]
                        for h in range(2):
                            nc.tensor.matmul(
                                pro[:, h * 64:(h + 1) * 64],
                                lhsT=wro_sb[kt // 16][
                                    :, (kt % 16) * 256 + h * 128:
                                    (kt % 16) * 256 + (h + 1) * 128],
                                rhs=rhs,
                                start=(i == 0 and h == 0),
                                stop=(i == KT - 1 and h == 1),
                                skip_group_check=True,
                            )
                    ro_emitted = min(ro_emitted + n_kts, KT)

                # L2: all hr projections first, then the whole mo=0 pass
                # before mo=1; fo=0 readout k-tiles ride between the mo=1
                # groups once the mo=0 h2 evictions have had time to land.
                all_hrs = [emit_hr(grp) for grp in range(GROUPS)]
                # pre-load the readout biases into pt (broadcast via rank-1
                # matmul); the final transposes accumulate onto them
                nc.tensor.matmul(pt[:], lhsT=ones_sb[:, 0:64], rhs=brow_sb,
                                 start=True, stop=False, skip_group_check=True)
                for grp in range(GROUPS):
                    emit_fm(grp, 0, all_hrs[grp])
                for grp in range(GROUPS):
                    emit_fm(grp, 1, all_hrs[grp])
                    if grp >= 2:
                        emit_ro(11)
                emit_ro(KT)
            # [128 lat, 64 g] -> [64 g, 256 lat]: one Copy-ACT applies the
            # 1/S3 scale, then two PE-transposes accumulate onto the
            # pre-loaded biases in pt; DVE copies the finished block out.
            mulv_sb = const.tile([128, 128], F32, tag="mulv_sb")
            nc.scalar.activation(mulv_sb[:], pro[:],
                                 mybir.ActivationFunctionType.Copy,
                                 scale=1.0 / S3)
            for h in range(2):
                nc.tensor.matmul(
                    pt[:, h * 128:(h + 1) * 128],
                    lhsT=mulv_sb[:, h * 64:(h + 1) * 64],
                    rhs=ident_sb,
                    is_transpose=True,
                    start=False, stop=(h == 1),
                    skip_group_check=True,
                )
            out_sb = const.tile([G_PER, 256], F32, tag="out_sb")
            nc.vector.tensor_copy(out_sb[:], pt[:])
            nc.sync.dma_start(out[:], out_sb[:])

    nc.compile()
    return nc


def _get_program():
    global _PROGRAM
    if _PROGRAM is None:
        _PROGRAM = _build_program()
    return _PROGRAM


def make_in_maps(x, W1_rel, W1_root, b1, W2_rel, W2_root, b2,
                 Wmu, bmu, Wlv, blv, edge_index, batch):
    """Host-side shard + layout prep. Returns per-core input dicts."""
    x = np.asarray(x, dtype=np.float32)
    edge_index = np.asarray(edge_index)

    b12 = np.stack(
        [np.asarray(b1)[0:128], np.asarray(b1)[128:256],
         np.asarray(b2)[0:128], np.asarray(b2)[128:256]], axis=1
    ).astype(np.float32)
    w1_pack = np.concatenate(
        [np.concatenate([np.asarray(W1_rel).T, np.asarray(W1_root).T],
                        axis=1).astype(BF16),
         np.ascontiguousarray(b12).view(BF16)], axis=1)
    w2rT = np.asarray(W2_rel).T.astype(np.float32)
    w2tT = np.asarray(W2_root).T.astype(np.float32)
    # readout biases as a bf16 [1, 256] row (bmu | blv) on partition 0; a
    # rank-1 matmul broadcasts them into the output psum before the final
    # transposes accumulate on top
    brow = np.zeros((128, 256), BF16)
    brow[0] = np.concatenate([np.asarray(bmu), np.asarray(blv)]).astype(BF16)
    w2 = np.concatenate(
        [np.concatenate([w2rT[0:128], w2tT[0:128]], axis=1).astype(BF16),
         np.concatenate([w2rT[128:256], w2tT[128:256]], axis=1).astype(BF16),
         np.ascontiguousarray(np.eye(128, dtype=np.float32)).view(BF16),
         brow], axis=1)
    # readout weights: e3m4, scaled by S3, laid out [p, kt, h, l'] so the
    # [128, 128] chunk for (kt, latent-half h) is a stationary lhsT
    wro_cat = np.concatenate([np.asarray(Wmu).T, np.asarray(Wlv).T], axis=1)
    wro = np.ascontiguousarray(
        (wro_cat * S3).reshape(KT, 128, 2, 128).transpose(1, 0, 2, 3)
        .reshape(128, KT * 256)
    ).astype(F8E3)

    # Dense per-2-graph-block adjacency counts: A[blk][s, d] = #edges s->d.
    src = edge_index[0].astype(np.int64)
    dst = edge_index[1].astype(np.int64)
    blk = dst >> 7                       # 128 nodes per 2-graph block
    s_loc = src - (blk << 7)
    d_loc = dst - (blk << 7)
    # edges are intra-graph by construction; fail loudly rather than let a
    # cross-block index wrap around in np.add.at
    assert s_loc.min() >= 0 and s_loc.max() < 128, "edge crosses graph block"
    A = np.zeros((BS // 2, 128, 128), np.float32)
    np.add.at(A, (blk, s_loc, d_loc), 1.0)

    in_maps = []
    for c in range(N_CORES):
        xs = x[c * NODES_PER:(c + 1) * NODES_PER]
        xsT2 = np.ascontiguousarray(xs.T) * 2.0     # x carries a factor of 2
        xw = np.concatenate([w1_pack, xsT2[:, 0:512].astype(BF16)], axis=1)
        xf8 = xsT2[:, 512:NODES_PER].astype(F8E3)
        Ac = A[c * BLOCKS:(c + 1) * BLOCKS]
        # nma: per block, [2*x node-major [128 node, 128 f] | a2t counts],
        # both fp8-e3m4 (counts <= 15 are exact in e3m4)
        xnm = xs.reshape(BLOCKS, 128, IN_F).transpose(1, 0, 2) * 2.0
        a2t = Ac.transpose(1, 0, 2)
        assert a2t.max() <= 15.0, "edge multiplicity exceeds e3m4 exact range"
        nma = np.ascontiguousarray(
            np.stack([xnm, a2t], axis=2).reshape(128, BLOCKS * 256)
        ).astype(F8E3)
        in_maps.append(dict(xw=xw, nma=nma, xf8=xf8, w2=w2, wro=wro))
    return in_maps


def kernel(**inputs):
    from concourse.bass_utils import run_bass_kernel_spmd

    nc = _get_program()
    in_maps = make_in_maps(**inputs)
    res = run_bass_kernel_spmd(nc, in_maps, list(range(N_CORES)))
    outs = np.concatenate(
        [res.results[c]["out"] for c in range(N_CORES)], axis=0)  # [512, 256]
    mu = np.ascontiguousarray(outs[:, :LAT]).astype(np.float32)
    logvar = np.ascontiguousarray(outs[:, LAT:]).astype(np.float32)
    return mu, logvar



# revision 38
# speedup vs baseline: 1.0127x; 1.0127x over previous
"""Trainium2 Bass kernel for nn_Encoder_conv_mlp (GNN message passing encoder).

Reference computation (per graph batch):
    h1 = relu(segsum(x[src]->dst) @ W1_rel.T + x @ W1_root.T + b1)
    h2 = relu(segsum(h1[src]->dst) @ W2_rel.T + h1 @ W2_root.T + b2)
    hb = h2.reshape(bs, 64*256)
    mu = hb @ Wmu.T + bmu ; logvar = hb @ Wlv.T + blv

Sharding: data-parallel over graphs. 512 graphs / 8 cores = 64 graphs
(4096 nodes, 65536 edges) per core. Edges never cross graphs, so each
core is fully independent; weights are replicated and the host simply
concatenates the per-core [64, 256] outputs.

Message passing is done as dense matmuls: the host builds, for every
2-graph block (128 nodes), an adjacency count matrix A2T[s, d] =
#edges(src=s -> dst=d). On device, aggregation is A2T contracted over
the src-node partition dim. Two matmul "families" avoid all transposes:
  - activations stationary (lhsT) + weights moving  -> node-major out
  - weights stationary (lhsT) + activations moving  -> feature-major out
Layer outputs are kept feature-major; the rel-projection (node-major) is
an intermediate only. All matmul operands are bf16 (fp32 PSUM accum).

The [16384, 256] readout weight [Wmu.T | Wlv.T] (8.4 MB bf16 per core)
is prefetched into SBUF while the conv layers run. Inputs are loaded
into per-chunk SBUF tiles so compute starts as soon as its chunk lands
(whole-tile deps would stall the PE on the full transfer), and h1 is
split per (ko, group) so layer 2 pipelines behind layer 1.

Further scheduling details: the f32 biases and the w1 weights ride
packed inside the xw/w2 bf16 input tensors (f32 values via bitcast
views on device) so the serial per-DMA launch overhead is paid fewer
times and the first matmul's dependencies arrive in a single transfer;
a short stream of discarded warm-up matmuls keeps the PE clock ramp
(HAM) busy while the first input DMAs land; and layer 2 runs all
rel-projections first, then the whole mo=0 output pass before mo=1, so
h2's first feature half (which gates the readout) completes while the
PE still has a full pass of work queued.
"""
import sys

if "/opt/trn_rl_repo" not in sys.path:
    sys.path.insert(0, "/opt/trn_rl_repo")

import numpy as np
import ml_dtypes

N_NODES = 64
BS = 512
IN_F = 128
HID = 256
LAT = 128
N_CORES = 8
G_PER = BS // N_CORES          # 64 graphs per core
NODES_PER = G_PER * N_NODES    # 4096 nodes per core
BLOCKS = NODES_PER // 128      # 32 two-graph blocks per core
GROUPS = NODES_PER // 512      # 8 512-node groups per core
KT = (N_NODES * HID) // 128    # 128 readout contraction tiles

BF16 = ml_dtypes.bfloat16
F8E3 = ml_dtypes.float8_e3m4
S3 = 512.0          # wro is stored as fp8-e3m4 * S3; readout evicts with 1/S3

_PROGRAM = None


def _build_program():
    import concourse.bacc as bacc
    import concourse.mybir as mybir
    import concourse.tile as tile

    nc = bacc.Bacc("TRN2", target_bir_lowering=False, debug=False,
                   num_devices=N_CORES)
    BF = mybir.dt.bfloat16
    F32 = mybir.dt.float32
    E3 = mybir.dt.float8e3

    xw = nc.dram_tensor("xw", [128, 1032], BF, kind="ExternalInput").ap()
    # nma: per 2-graph block, [x node-major (128) | a2t counts (128)] pairs —
    # L1 aggregates x directly (agg-first), so each block's pair arrives in
    # one contiguous chunk.
    nma = nc.dram_tensor("nma", [128, BLOCKS * 256], E3, kind="ExternalInput").ap()
    # feature-major x for groups 1-7, fp8-e3m4 (x scaled by 2; group 0 rides
    # in the bf16 lead, also pre-scaled by 2)
    xf8 = nc.dram_tensor("xf8", [128, 3584], E3, kind="ExternalInput").ap()
    # w2 carries [W2 packs | 128x128 bf16 identity | bmu/blv per-latent f32]
    w2 = nc.dram_tensor("w2", [128, 1536], BF, kind="ExternalInput").ap()
    # readout weights in fp8-e3m4 (scaled by S3): halves the dominant DMA
    wro = nc.dram_tensor("wro", [128, KT * 256], E3, kind="ExternalInput").ap()
    out = nc.dram_tensor("out", [G_PER, 256], F32, kind="ExternalOutput").ap()

    Relu = mybir.ActivationFunctionType.Relu

    with tile.TileContext(nc) as tc:
        with (
            tc.tile_pool(name="const", bufs=1) as const,
            tc.tile_pool(name="hr", bufs=20) as hr_pool,
            tc.tile_pool(name="psum_hr", bufs=3, space="PSUM") as psum_hr,
            tc.tile_pool(name="psum_fm", bufs=3, space="PSUM") as psum_fm,
            tc.tile_pool(name="psum_ro", bufs=1, space="PSUM") as psum_ro,
            tc.tile_pool(name="psum_t", bufs=1, space="PSUM") as psum_t,
        ):
            # Per-chunk tiles so each consumer depends only on its chunk's DMA.
            lead_sb = const.tile([128, 1032], BF, tag="lead_sb")
            xT0b_sb = const.tile([128, 512], E3, tag="xT0b_sb")
            xT_sb = [const.tile([128, 1024], E3, name=f"xT{i}", tag=f"xT{i}")
                     for i in range(1, 4)]
            # nma per-group tiles; group 0 is split so block 0's (x_nm|a2t)
            # pair lands in the smallest possible first transfer.
            nm0a_sb = const.tile([128, 256], E3, tag="nm0a_sb")
            nm0b_sb = const.tile([128, 768], E3, tag="nm0b_sb")
            nm_sb = [const.tile([128, 1024], E3, name=f"nm{g}", tag=f"nm{g}")
                     for g in range(1, GROUPS)]
            w2_sb = const.tile([128, 1536], BF, tag="w2_sb")
            wro_sb = [const.tile([128, 4096], E3, name=f"wro{i}", tag=f"wro{i}") for i in range(8)]
            # h1 split per (ko, group) for L1->L2 pipelining; h2 per ko chunk.
            h1_sb = [[const.tile([128, 512], BF, name=f"h1_{ko}_{g}", tag=f"h1_{ko}_{g}")
                      for g in range(GROUPS)] for ko in range(2)]
            h2_sb = [const.tile([128, NODES_PER], BF, name=f"h2_{fo}", tag=f"h2_{fo}")
                     for fo in range(2)]

            def nm_chunk(b):           # (x_nm | a2t) [128, 256] pair, block b
                if b == 0:
                    return nm0a_sb[:, 0:256]
                if b < 4:
                    return nm0b_sb[:, (b - 1) * 256:b * 256]
                return nm_sb[b // 4 - 1][:, (b % 4) * 256:(b % 4 + 1) * 256]

            def x_nm_blk(b):           # node-major x block [128 node, 128 f]
                return nm_chunk(b)[:, 0:128]

            def a2t_blk(b):            # [128, 128] adjacency for block b
                return nm_chunk(b)[:, 128:256]

            # DMA issue order = priority order for the head of the kernel.
            # Block 0's aggregation pair goes first (it gates the very first
            # real matmul), then the lead transfer (w1 + biases + group 0's
            # feature-major x), then x/nma chunks in consumption order ahead
            # of w2 and the big readout-weight stream.
            nc.sync.dma_start(nm0a_sb[:], nma[:, 0:256])
            nc.sync.dma_start(nm0b_sb[:], nma[:, 256:1024])
            nc.sync.dma_start(lead_sb[:], xw[:, 0:1032])
            nc.sync.dma_start(nm_sb[0][:], nma[:, 1024:2048])
            nc.sync.dma_start(xT0b_sb[:], xf8[:, 0:512])
            for i in range(1, 4):
                nc.sync.dma_start(nm_sb[2 * i - 1][:],
                                  nma[:, 2 * i * 1024:(2 * i + 1) * 1024])
                nc.sync.dma_start(nm_sb[2 * i][:],
                                  nma[:, (2 * i + 1) * 1024:(2 * i + 2) * 1024])
                nc.sync.dma_start(xT_sb[i - 1][:],
                                  xf8[:, i * 1024 - 512:(i + 1) * 1024 - 512])
            nc.sync.dma_start(w2_sb[:], w2[:])
            # w1 + biases ride packed inside lead/w2 (bitcast views for f32)
            w1_sb = lead_sb[:, 0:520]
            b12_sb = lead_sb[:, 512:520].bitcast(F32)
            # [128, 128] f32 identity for the f32 PE-transposes
            ident_sb = w2_sb[:, 1024:1280].bitcast(F32)
            brow_sb = w2_sb[0:1, 1280:1536]             # [1, 256] bf16 bmu|blv
            for i in range(8):
                nc.sync.dma_start(wro_sb[i][:], wro[:, i * 4096:(i + 1) * 4096])

            # PE pre-warm: dummy matmuls on memset data keep the PE busy from
            # ~1.1us so the clock ramp (HAM) completes before the first real
            # matmul arrives behind the input DMAs (~3.3us); the count is
            # tuned so the warm stream ends just as the real one begins.
            # Results are discarded; the psum slot is reused by the readout.
            N_WARM = 16
            ones_sb = const.tile([1, 256], BF, tag="ones_sb")
            nc.vector.memset(ones_sb[:], 1.0)
            # ro_big hosts the warmup target, then the readout accumulator —
            # one psum bank serves both phases.
            ro_big = psum_ro.tile([128, 128], F32, tag="pro")
            warm = ro_big[:, 0:128]
            # pt gets its own bank: the per-latent biases are pre-loaded into
            # it by a rank-1 matmul mid-kernel, and the final transposes
            # accumulate on top — so it must not share a psum zero-region
            # with the readout accumulator.
            pt = psum_t.tile([G_PER, 256], F32, tag="pt")
            for i in range(N_WARM):
                nc.tensor.matmul(warm[:], lhsT=ones_sb[:, 128:256],
                                 rhs=ones_sb[:, 0:128],
                                 start=(i == 0), stop=(i == N_WARM - 1))

            def x_cols(c0, c1):        # feature-major x slice [128, c1-c0]
                g = c0 // 512
                if g == 0:
                    assert c1 <= 512
                    return lead_sb[:, 520 + c0:520 + c1]
                if g == 1:
                    assert c1 <= 1024
                    return xT0b_sb[:, c0 - 512:c1 - 512]
                i = (c0 - 1024) // 1024
                assert c1 - 1024 <= (i + 1) * 1024
                return xT_sb[i][:, c0 - 1024 - i * 1024:c1 - 1024 - i * 1024]

            # ---- Layer 1: aggregate-first ----
            # agg_x = A @ x per block (x node-major as stationary, a2t
            # moving), evicted to SBUF; then h1 = relu(W1relT-proj(agg_x) +
            # W1rootT-proj(x_fm) + b1) lands feature-major in one psum pass.
            # Aggregating the 128-wide x (instead of the 256-wide x@W1rel)
            # halves L1's aggregation matmul rows vs project-then-aggregate.
            # Software-pipelined: the aggregation for group g+1 is emitted
            # before group g's projection pass, so the PE chews on agg(g+1)
            # while g's psum->SBUF eviction (DVE) is still in flight.
            def emit_agg(grp):
                pag = psum_hr.tile([128, 512], F32, name="ph", tag="ph")
                for blk in range(4):
                    b = grp * 4 + blk
                    nc.tensor.matmul(
                        pag[:, blk * 128:(blk + 1) * 128],
                        lhsT=x_nm_blk(b), rhs=a2t_blk(b),
                        start=True, stop=True, skip_group_check=True,
                    )
                aggx = hr_pool.tile([128, 512], BF, name="aggx", tag="aggx")
                nc.vector.tensor_copy(aggx[:], pag[:])
                return aggx

            def emit_l1fm(grp, aggx):
                xg = x_cols(grp * 512, (grp + 1) * 512)
                for mo in range(2):
                    pf = psum_fm.tile([128, 512], F32, name="pf", tag="pf")
                    nc.tensor.matmul(
                        pf[:], lhsT=w1_sb[:, mo * 128:(mo + 1) * 128],
                        rhs=aggx[:], start=True, stop=False,
                        skip_group_check=True,
                    )
                    nc.tensor.matmul(
                        pf[:], lhsT=w1_sb[:, 256 + mo * 128:256 + (mo + 1) * 128],
                        rhs=xg, start=False, stop=True,
                        skip_group_check=True,
                    )
                    if mo == 0:
                        nc.scalar.activation(
                            h1_sb[mo][grp][:], pf[:], Relu,
                            bias=b12_sb[:, mo:mo + 1],
                        )
                    else:
                        nc.vector.tensor_scalar(
                            h1_sb[mo][grp][:], pf[:],
                            scalar1=b12_sb[:, mo:mo + 1], scalar2=0.0,
                            op0=mybir.AluOpType.add, op1=mybir.AluOpType.max,
                        )

            aggxs = [emit_agg(0), emit_agg(1)]
            for grp in range(GROUPS):
                if grp + 2 < GROUPS:
                    aggxs.append(emit_agg(grp + 2))
                emit_l1fm(grp, aggxs[grp])

            # ---- Layer 2 ----
            for layer in [1]:
                n_ko = 2
                act_cols = lambda ko, c0, c1: (
                    h1_sb[ko][c0 // 512][:, c0 % 512:c0 % 512 + (c1 - c0)])
                w_rel = lambda ko: w2_sb[:, ko * 512:ko * 512 + 256]
                w_root = lambda ko, mo: w2_sb[:, ko * 512 + 256 + mo * 128:
                                              ko * 512 + 256 + (mo + 1) * 128]
                bias_col = 2

                def emit_hr(grp):
                    # two blocks share one [128,512] psum tile (same bank
                    # footprint as a padded [128,256]) so one DVE copy evicts
                    # both -> half the copy count, ~4us less DVE busy
                    hrs = []
                    for pair in range(2):
                        ph = psum_hr.tile([128, 512], F32, name="ph", tag="ph")
                        for sub in range(2):
                            b = grp * 4 + pair * 2 + sub
                            for ko in range(n_ko):
                                nc.tensor.matmul(
                                    ph[:, sub * 256:(sub + 1) * 256],
                                    lhsT=act_cols(ko, b * 128, (b + 1) * 128),
                                    rhs=w_rel(ko),
                                    start=(ko == 0), stop=(ko == n_ko - 1),
                                    skip_group_check=True,
                                )
                        hr = hr_pool.tile([128, 512], BF)
                        if pair == 0:
                            nc.vector.tensor_copy(hr[:], ph[:])
                        else:
                            nc.scalar.activation(
                                hr[:], ph[:],
                                mybir.ActivationFunctionType.Copy)
                        hrs.append(hr)
                    return hrs

                def emit_fm(grp, mo, hrs):
                    pf = psum_fm.tile([128, 512], F32, name="pf", tag="pf")
                    for ko in range(n_ko):
                        nc.tensor.matmul(
                            pf[:],
                            lhsT=w_root(ko, mo),
                            rhs=act_cols(ko, grp * 512, (grp + 1) * 512),
                            start=(ko == 0), stop=False,
                            skip_group_check=True,
                        )
                    for blk in range(4):
                        b = grp * 4 + blk
                        nc.tensor.matmul(
                            pf[:, blk * 128:(blk + 1) * 128],
                            lhsT=hrs[blk // 2][:, (blk % 2) * 256 + mo * 128:
                                               (blk % 2) * 256 + (mo + 1) * 128],
                            rhs=a2t_blk(b),
                            start=False, stop=(blk == 3),
                            skip_group_check=True,
                        )
                    dst = h2_sb[mo][:, grp * 512:(grp + 1) * 512]
                    nc.scalar.activation(
                        dst, pf[:], Relu,
                        bias=b12_sb[:, bias_col + mo:bias_col + mo + 1],
                        scale=0.5,
                    )

                # ---- Readout accumulation, interleaved with L2's tail ----
                # Orientation: latent on the PSUM partition dim (128, full)
                # and graphs on the free dim (64) — half the matmul rows of
                # the graphs-on-partition orientation. wro is the fp8-e3m4
                # stationary operand; h2 (bf16) moves. pro[:, 0:64]
                # accumulates mu, pro[:, 64:128] logvar; one group spans both.
                pro = ro_big[:, 0:128]
                ro_emitted = 0

                def emit_ro(n_kts):
                    # fo=0 k-tiles first (they only need the mo=0 pass of
                    # h2), fo=1 after; interleaving fo=0 emission into the
                    # mo=1 projection pass keeps the PE fed while the last
                    # h2 evictions drain.
                    nonlocal ro_emitted
                    kts = [kt for kt in range(KT) if kt % 2 == 0] + \
                          [kt for kt in range(KT) if kt % 2 == 1]
                    for i in range(ro_emitted, min(ro_emitted + n_kts, KT)):
                        kt = kts[i]
                        n, fo = kt // 2, kt % 2
                        rhs = h2_sb[fo][:, n:n + (G_PER - 1) * N_NODES + 1:
                                        N_NODES]
                        for h in range(2):
                            nc.tensor.matmul(
                                pro[:, h * 64:(h + 1) * 64],
                                lhsT=wro_sb[kt // 16][
                                    :, (kt % 16) * 256 + h * 128:
                                    (kt % 16) * 256 + (h + 1) * 128],
                                rhs=rhs,
                                start=(i == 0 and h == 0),
                                stop=(i == KT - 1 and h == 1),
                                skip_group_check=True,
                            )
                    ro_emitted = min(ro_emitted + n_kts, KT)

                # L2: all hr projections first, then the whole mo=0 pass
                # before mo=1; fo=0 readout k-tiles ride between the mo=1
                # groups once the mo=0 h2 evictions have had time to land.
                all_hrs = [emit_hr(grp) for grp in range(GROUPS)]
                # pre-load the readout biases into pt (broadcast via rank-1
                # matmul); the final transposes accumulate onto them
                nc.tensor.matmul(pt[:], lhsT=ones_sb[:, 0:64], rhs=brow_sb,
                                 start=True, stop=False, skip_group_check=True)
                for grp in range(GROUPS):
                    emit_fm(grp, 0, all_hrs[grp])
                for grp in range(GROUPS):
                    emit_fm(grp, 1, all_hrs[grp])
                    if grp >= 2:
                        emit_ro(9)
                emit_ro(KT)
            # [128 lat, 64 g] -> [64 g, 256 lat]: one Copy-ACT applies the
            # 1/S3 scale, then two PE-transposes accumulate onto the
            # pre-loaded biases in pt; DVE copies the finished block out.
            mulv_sb = const.tile([128, 128], F32, tag="mulv_sb")
            nc.scalar.activation(mulv_sb[:], pro[:],
                                 mybir.ActivationFunctionType.Copy,
                                 scale=1.0 / S3)
            for h in range(2):
                nc.tensor.matmul(
                    pt[:, h * 128:(h + 1) * 128],
                    lhsT=mulv_sb[:, h * 64:(h + 1) * 64],
                    rhs=ident_sb,
                    is_transpose=True,
                    start=False, stop=(h == 1),
                    skip_group_check=True,
                )
            out_sb = const.tile([G_PER, 256], F32, tag="out_sb")
            nc.vector.tensor_copy(out_sb[:], pt[:])
            nc.sync.dma_start(out[:], out_sb[:])

    nc.compile()
    return nc


def _get_program():
    global _PROGRAM
    if _PROGRAM is None:
        _PROGRAM = _build_program()
    return _PROGRAM


def make_in_maps(x, W1_rel, W1_root, b1, W2_rel, W2_root, b2,
                 Wmu, bmu, Wlv, blv, edge_index, batch):
    """Host-side shard + layout prep. Returns per-core input dicts."""
    x = np.asarray(x, dtype=np.float32)
    edge_index = np.asarray(edge_index)

    b12 = np.stack(
        [2.0 * np.asarray(b1)[0:128], 2.0 * np.asarray(b1)[128:256],
         np.asarray(b2)[0:128], np.asarray(b2)[128:256]], axis=1
    ).astype(np.float32)
    w1_pack = np.concatenate(
        [np.concatenate([np.asarray(W1_rel).T, np.asarray(W1_root).T],
                        axis=1).astype(BF16),
         np.ascontiguousarray(b12).view(BF16)], axis=1)
    w2rT = np.asarray(W2_rel).T.astype(np.float32)
    w2tT = np.asarray(W2_root).T.astype(np.float32)
    # readout biases as a bf16 [1, 256] row (bmu | blv) on partition 0; a
    # rank-1 matmul broadcasts them into the output psum before the final
    # transposes accumulate on top
    brow = np.zeros((128, 256), BF16)
    brow[0] = np.concatenate([np.asarray(bmu), np.asarray(blv)]).astype(BF16)
    w2 = np.concatenate(
        [np.concatenate([w2rT[0:128], w2tT[0:128]], axis=1).astype(BF16),
         np.concatenate([w2rT[128:256], w2tT[128:256]], axis=1).astype(BF16),
         np.ascontiguousarray(np.eye(128, dtype=np.float32)).view(BF16),
         brow], axis=1)
    # readout weights: e3m4, scaled by S3, laid out [p, kt, h, l'] so the
    # [128, 128] chunk for (kt, latent-half h) is a stationary lhsT
    wro_cat = np.concatenate([np.asarray(Wmu).T, np.asarray(Wlv).T], axis=1)
    wro = np.ascontiguousarray(
        (wro_cat * S3).reshape(KT, 128, 2, 128).transpose(1, 0, 2, 3)
        .reshape(128, KT * 256)
    ).astype(F8E3)

    # Dense per-2-graph-block adjacency counts: A[blk][s, d] = #edges s->d.
    src = edge_index[0].astype(np.int64)
    dst = edge_index[1].astype(np.int64)
    blk = dst >> 7                       # 128 nodes per 2-graph block
    s_loc = src - (blk << 7)
    d_loc = dst - (blk << 7)
    # edges are intra-graph by construction; fail loudly rather than let a
    # cross-block index wrap around in np.add.at
    assert s_loc.min() >= 0 and s_loc.max() < 128, "edge crosses graph block"
    A = np.zeros((BS // 2, 128, 128), np.float32)
    np.add.at(A, (blk, s_loc, d_loc), 1.0)

    in_maps = []
    for c in range(N_CORES):
        xs = x[c * NODES_PER:(c + 1) * NODES_PER]
        xsT2 = np.ascontiguousarray(xs.T) * 2.0     # x carries a factor of 2
        xw = np.concatenate([w1_pack, xsT2[:, 0:512].astype(BF16)], axis=1)
        xf8 = xsT2[:, 512:NODES_PER].astype(F8E3)
        Ac = A[c * BLOCKS:(c + 1) * BLOCKS]
        # nma: per block, [2*x node-major [128 node, 128 f] | a2t counts],
        # both fp8-e3m4 (counts <= 15 are exact in e3m4)
        xnm = xs.reshape(BLOCKS, 128, IN_F).transpose(1, 0, 2) * 2.0
        a2t = Ac.transpose(1, 0, 2)
        assert a2t.max() <= 15.0, "edge multiplicity exceeds e3m4 exact range"
        nma = np.ascontiguousarray(
            np.stack([xnm, a2t], axis=2).reshape(128, BLOCKS * 256)
        ).astype(F8E3)
        in_maps.append(dict(xw=xw, nma=nma, xf8=xf8, w2=w2, wro=wro))
    return in_maps


def kernel(**inputs):
    from concourse.bass_utils import run_bass_kernel_spmd

    nc = _get_program()
    in_maps = make_in_maps(**inputs)
    res = run_bass_kernel_spmd(nc, in_maps, list(range(N_CORES)))
    outs = np.concatenate(
        [res.results[c]["out"] for c in range(N_CORES)], axis=0)  # [512, 256]
    mu = np.ascontiguousarray(outs[:, :LAT]).astype(np.float32)
    logvar = np.ascontiguousarray(outs[:, LAT:]).astype(np.float32)
    return mu, logvar

